# revision 22
# baseline (speedup 1.0000x reference)
"""Cross-attention Trainium2 kernel, 8-core data-parallel.

Problem (hardcoded): B=4, NQ=4096, NK=1024, QD=1024, CD=768, H=16, HD=64.
  out = softmax((x@Wq) @ (ctx@Wk)^T / sqrt(HD)) @ (ctx@Wv) @ Wo + bo

Sharding: pure data-parallel. 8 cores = 4 batches x 2 NQ-halves of 2048
query rows. Each core redundantly computes K/V projections for its batch
(cheap) and needs no collectives.

Per-core layout: every matmul keeps its contraction dim on SBUF
partitions, so the chain is computed fully "transposed":
  QT[qd,q]   = Wq^T-contract(xT)        (x pre-transposed on host)
  KT[qd,k]   = Wk^T-contract(ctxT)
  V'[k,qd+1] = ctx@Wv with a ones column appended per head
  ST[k,q]    = per head: KT_h^T-slices @ QT_h (row-tiled 2 heads/matmul)
  ET         = exp(ST/8): split between ACT (exact LUT exp) and DVE
               (Schraudolph bf16 bit-trick: tensor_scalar mult+add with
               int16 write = bf16 bit pattern of 2^x)
  O'T[65,q]  = V'_h^T @ ET_h            (row 64 = softmax denominators)
  attnT      = O'T[0:64] * recip(denoms): per head a [1,512] denominator
               copy, reciprocal_approx_fast, gpsimd partition broadcast,
               then one DVE multiply straight from PSUM
  out[q,od]  = attnT^T @ Wo + bo        (bf16 out, f32-cast on host)

vs baseline: exp work split ACT/DVE (ACT exp was a 268us serial
bottleneck); K/Q/V projections pipelined into the attention loop as PE
filler (no serial prologue); wo staged late + xT streamed in halves to
fit SBUF; ET tiles split in kc-quads; exp table preloaded; psO=3/psX=1
PSUM split; bf16 output DMA. Per-matmul LDWEIGHTS (ldw-opt disabled in
this toolchain) sets the PE floor at ~385us busy; measured span 463-486ns
run-to-run.
"""

import numpy as np

B, NQ, NK = 4, 4096, 1024
QD, CD, H = 1024, 768, 16
HD = QD // H
SCALE = HD ** -0.5
NQL = NQ // 2          # query rows per core
N_CORES = 8

# Schraudolph bf16 exp: bits = round(x*SCALE*log2e*128 + 127*128 + c)
EXP_K1 = float(SCALE * 128.0 / np.log(2.0))
EXP_K2 = float(127 * 128 - 5.25)
DVE_KC = 2             # of 8 kc chunks, how many use the DVE approx exp


def build_bass():
    """Build the per-core Bass graph (SPMD, identical on all 8 cores)."""
    import concourse.bass as bass
    import concourse.tile as tile
    from concourse import bacc, mybir

    f32 = mybir.dt.float32
    bf16 = mybir.dt.bfloat16
    i16 = mybir.dt.int16
    EXP = mybir.ActivationFunctionType.Exp
    MULT = mybir.AluOpType.mult
    ADD = mybir.AluOpType.add

    nc = bacc.Bacc()

    xT_h = nc.dram_tensor("xT", (QD, NQL), bf16, kind="ExternalInput")
    ctxT_h = nc.dram_tensor("ctxT", (CD, NK), bf16, kind="ExternalInput")
    wq_h = nc.dram_tensor("wq", (QD, QD), bf16, kind="ExternalInput")
    wk_h = nc.dram_tensor("wk", (CD, QD), bf16, kind="ExternalInput")
    wv_h = nc.dram_tensor("wv", (CD, QD), bf16, kind="ExternalInput")
    wo_h = nc.dram_tensor("wo", (QD, QD), bf16, kind="ExternalInput")
    bo_h = nc.dram_tensor("bo", (1, QD), bf16, kind="ExternalInput")
    out_h = nc.dram_tensor("out", (NQL, QD), bf16, kind="ExternalOutput")

    xT_d = xT_h[:].rearrange("(c p) n -> p c n", p=128)       # [128, 8, 2048]
    ctxT_d = ctxT_h[:].rearrange("(c p) n -> p c n", p=128)   # [128, 6, 1024]
    wq_d = wq_h[:].rearrange("(c p) m -> p c m", p=128)
    wk_d = wk_h[:].rearrange("(c p) m -> p c m", p=128)
    wv_d = wv_h[:].rearrange("(c p) m -> p c m", p=128)
    wo_d = wo_h[:].rearrange("(c p) m -> p c m", p=128)
    out_d = out_h[:].rearrange("(t p) n -> p t n", p=128)     # [128, 16, 1024]

    KC_Q = QD // 128   # 8
    KC_C = CD // 128   # 6
    NKC = NK // 128    # 8
    HP = H // 2        # 8

    with tile.TileContext(nc) as tc:
        _pp_cm = tc.tile_pool(name="persist", bufs=1)
        pp = _pp_cm.__enter__()
        qt_sb = pp.tile([128, KC_Q, NQL], bf16)        # 32 KB/p
        kt_sb = pp.tile([128, KC_Q, NK], bf16)         # 16 KB/p
        vp_sb = pp.tile([128, NKC, H, HD + 1], bf16)   # 16.25 KB/p
        attnT_sb = pp.tile([128, KC_Q, NQL], bf16)     # 32 KB/p
        bo_bc = pp.tile([128, QD], bf16)               # 2 KB/p
        bo_sb = pp.tile([1, QD], bf16)
        ones_sb = pp.tile([1, 128], bf16)

        _ps_cm = tc.tile_pool(name="psum", bufs=1, space=bass.MemorySpace.PSUM)
        psp = _ps_cm.__enter__()

        def psS():
            return psp.tile([128, 1024], f32, tag="psS", bufs=2, name="psS")

        def psO():
            return psp.tile([HD + 1, 512], f32, tag="psO", bufs=3, name="psO")

        def psX():
            return psp.tile([128, 512], f32, tag="psX", bufs=1, name="psX")

        _xq_cm = tc.tile_pool(name="xq", bufs=1)
        xq = _xq_cm.__enter__()
        wq_sb = xq.tile([128, KC_Q, QD], bf16)         # 16 KB/p
        _wo_cm = None
        wo_holder = {}

        def evict(dst, src, parity):
            if parity % 2 == 0:
                nc.vector.tensor_copy(dst, src)
            else:
                nc.scalar.copy(dst, src)

        def k_proj(mo):
            for nk in range(NK // 512):
                ps = psX()
                for c in range(KC_C):
                    nc.tensor.matmul(
                        ps[:],
                        wk_sb[:, c, mo * 128:(mo + 1) * 128],
                        ctxT_sb[:, c, nk * 512:(nk + 1) * 512],
                        start=(c == 0), stop=(c == KC_C - 1),
                    )
                evict(kt_sb[:, mo, nk * 512:(nk + 1) * 512], ps[:], mo + nk)

        def v_proj(nv):
            for ko in range(NKC):
                ps = psX()
                for c in range(KC_C):
                    nc.tensor.matmul(
                        ps[:],
                        ctxT_sb[:, c, ko * 128:(ko + 1) * 128],
                        wv_sb[:, c, nv * 512:(nv + 1) * 512],
                        start=(c == 0), stop=(c == KC_C - 1),
                    )
                evict(vp_sb[:, ko, nv * 8:(nv + 1) * 8, 0:HD],
                      ps[:].rearrange("p (h d) -> p h d", h=8), ko + nv)

        def q_proj(mo, half):
            # qt_sb[:, mo, half*1024 : +1024]
            xh = xh_holder[half]
            for nq in range(2):
                ps = psX()
                for c in range(KC_Q):
                    nc.tensor.matmul(
                        ps[:],
                        wq_sb[:, c, mo * 128:(mo + 1) * 128],
                        xh[:, c, nq * 512:(nq + 1) * 512],
                        start=(c == 0), stop=(c == KC_Q - 1),
                    )
                dst = qt_sb[:, mo,
                            half * 1024 + nq * 512:half * 1024 + (nq + 1) * 512]
                evict(dst, ps[:], mo + nq)

        xh_holder = {}

        with (
            tc.tile_pool(name="xin", bufs=1) as pxin,
            tc.tile_pool(name="et", bufs=3) as pe_pool,
            tc.tile_pool(name="rsmall", bufs=1) as prs,
            tc.tile_pool(name="yout", bufs=2) as py,
        ):
            def y_cols(mo):
                wo_sb = wo_holder["wo"]
                for no in range(QD // 512):
                    ps = psX()
                    for c in range(KC_Q):
                        nc.tensor.matmul(
                            ps[:],
                            attnT_sb[:, c, mo * 128:(mo + 1) * 128],
                            wo_sb[:, c, no * 512:(no + 1) * 512],
                            start=(c == 0), stop=(c == KC_Q - 1),
                        )
                    y = py.tile([128, 512], bf16, tag="y", name="y")
                    nc.vector.tensor_add(
                        y[:], ps[:], bo_bc[:, no * 512:(no + 1) * 512])
                    nc.sync.dma_start(
                        out_d[:, mo, no * 512:(no + 1) * 512], y[:])

            def attention(qt2, hp):
                h0, h1 = 2 * hp, 2 * hp + 1
                q0 = qt2 * 1024
                # denominator rows parked at partitions 0/32/64/96 (32-aligned
                # base requirement); reciprocal runs over all 128 rows, the
                # unwritten rows are garbage and unused.
                for qh in range(2):
                    qsl = slice(q0 + qh * 512, q0 + (qh + 1) * 512)
                    etq = []
                    for kcq in range(2):
                        et = pe_pool.tile([128, 4, 1024], bf16, tag="etq",
                                          name="etq")
                        etq.append(et)
                        for k4 in range(4):
                            kc = kcq * 4 + k4
                            ks = slice(kc * 128, (kc + 1) * 128)
                            ps = psS()
                            nc.tensor.matmul(
                                ps[:, 0:512],
                                kt_sb[0:64, hp, ks], qt_sb[0:64, hp, qsl],
                                start=True, stop=True, tile_position=(0, 0),
                            )
                            nc.tensor.matmul(
                                ps[:, 512:1024],
                                kt_sb[64:128, hp, ks], qt_sb[64:128, hp, qsl],
                                start=True, stop=True, tile_position=(64, 0),
                            )
                            if kc < NKC - DVE_KC:
                                nc.scalar.activation(
                                    et[:, k4, :], ps[:], EXP, scale=SCALE)
                            else:
                                nc.vector.tensor_scalar(
                                    et[:, k4, :].bitcast(i16), ps[:],
                                    EXP_K1, EXP_K2, MULT, ADD)
                    for h_i, h in enumerate((h0, h1)):
                        po = psO()
                        esl = slice(h_i * 512, (h_i + 1) * 512)
                        for kc in range(NKC):
                            nc.tensor.matmul(
                                po[:], vp_sb[:, kc, h, :],
                                etq[kc // 4][:, kc % 4, esl],
                                start=(kc == 0), stop=(kc == NKC - 1),
                            )
                        # normalize: denom row -> recip -> broadcast -> mult
                        sums = prs.tile([1, 512], f32, tag="sums", bufs=2,
                                        name="sums")
                        nc.vector.tensor_copy(sums[:], po[HD:HD + 1, :])
                        rf = prs.tile([1, 512], f32, tag="rf", bufs=2,
                                      name="rf")
                        nc.vector.reciprocal_approx_fast(rf[:], sums[:])
                        rs = prs.tile([64, 512], f32, tag="rs", bufs=2,
                                      name="rs")
                        nc.gpsimd.partition_broadcast(rs[:], rf[:])
                        prow = slice(h_i * 64, h_i * 64 + 64)
                        nc.vector.tensor_mul(
                            attnT_sb[prow, hp, qsl], po[0:HD, :], rs[:])

            with tc.tile_pool(name="kvin", bufs=1) as pkv:
                ctxT_sb = pkv.tile([128, KC_C, NK], bf16)      # 12 KB/p
                wk_sb = pkv.tile([128, KC_C, QD], bf16)
                wv_sb = pkv.tile([128, KC_C, QD], bf16)

                # staged DMA
                nc.sync.dma_start(ctxT_sb[:, :, 0:512], ctxT_d[:, :, 0:512])
                nc.sync.dma_start(wk_sb[:, :, 0:256], wk_d[:, :, 0:256])
                nc.sync.dma_start(bo_sb[:], bo_h[:])
                nc.sync.dma_start(ctxT_sb[:, :, 512:1024], ctxT_d[:, :, 512:1024])
                nc.sync.dma_start(wk_sb[:, :, 256:1024], wk_d[:, :, 256:1024])
                nc.sync.dma_start(wv_sb[:], wv_d)
                xh0 = pxin.tile([128, KC_Q, NQL // 2], bf16, tag="xh",
                                name="xh0")
                xh_holder[0] = xh0
                nc.sync.dma_start(xh0[:], xT_d[:, :, 0:NQL // 2])
                nc.sync.dma_start(wq_sb[:], wq_d)

                nc.vector.memset(ones_sb[:], 1.0)
                nc.vector.memset(vp_sb[:, :, :, HD], 1.0)
                # preload the exp table set off the critical path
                nc.scalar.activation(bo_bc[0:1, 0:1], ones_sb[0:1, 0:1], EXP)
                for no in range(QD // 512):
                    psb = psX()
                    nc.tensor.matmul(psb[:], ones_sb[:],
                                     bo_sb[0:1, no * 512:(no + 1) * 512],
                                     start=True, stop=True)
                    nc.scalar.copy(bo_bc[:, no * 512:(no + 1) * 512], psb[:])

                # pipelined prologue: K/V first (their DMAs land first),
                # Q as soon as xT/wq arrive, attention chases per head-pair
                k_proj(0)
                k_proj(1)
                v_proj(0)
                k_proj(2)
                k_proj(3)
                q_proj(0, 0)
                q_proj(1, 0)
                attention(0, 0)
                k_proj(4)
                q_proj(2, 0)
                attention(0, 1)
                k_proj(5)
                q_proj(3, 0)
                v_proj(1)
                attention(0, 2)
                k_proj(6)
                q_proj(4, 0)
                attention(0, 3)
                k_proj(7)
                for mo in range(5, 8):
                    q_proj(mo, 0)

            # ctx/wk/wv released; stage wo into freed space
            _wo_cm = tc.tile_pool(name="wop", bufs=1)
            pwo = _wo_cm.__enter__()
            wo_sb = pwo.tile([128, KC_Q, QD], bf16)
            wo_holder["wo"] = wo_sb
            nc.sync.dma_start(wo_sb[:], wo_d)

            attention(0, 4)
            xh1 = pxin.tile([128, KC_Q, NQL // 2], bf16, tag="xh", name="xh1")
            xh_holder[1] = xh1
            nc.sync.dma_start(xh1[:], xT_d[:, :, NQL // 2:NQL])
            attention(0, 5)
            for mo in range(4):
                q_proj(mo, 1)
            attention(0, 6)
            for mo in range(4, 8):
                q_proj(mo, 1)
            attention(0, 7)
            for mo in range(8):
                y_cols(mo)
            for hp in range(HP):
                attention(1, hp)
            for mo in range(8, 16):
                y_cols(mo)

            _wo_cm.__exit__(None, None, None)

        _xq_cm.__exit__(None, None, None)
        _ps_cm.__exit__(None, None, None)
        _pp_cm.__exit__(None, None, None)

    nc.finalize()
    return nc


def make_in_maps(x, context, Wq, Wk, Wv, Wo, bo):
    """Host-side sharding + layout prep: transpose and cast to bf16."""
    import ml_dtypes
    bf16 = ml_dtypes.bfloat16

    x = np.asarray(x, np.float32)
    context = np.asarray(context, np.float32)
    wq = np.asarray(Wq, np.float32).astype(bf16)
    wk = np.asarray(Wk, np.float32).astype(bf16)
    wv = np.asarray(Wv, np.float32).astype(bf16)
    wo = np.asarray(Wo, np.float32).astype(bf16)
    bo = np.asarray(bo, np.float32).reshape(1, QD).astype(bf16)

    in_maps = []
    for c in range(N_CORES):
        b, half = c // 2, c % 2
        xs = x[b, half * NQL:(half + 1) * NQL, :]           # [2048, 1024]
        in_maps.append({
            "xT": np.ascontiguousarray(xs.T).astype(bf16),   # [1024, 2048]
            "ctxT": np.ascontiguousarray(context[b].T).astype(bf16),
            "wq": wq, "wk": wk, "wv": wv, "wo": wo, "bo": bo,
        })
    return in_maps


_NC_CACHE = {}


def kernel(x, context, Wq, Wk, Wv, Wo, bo, _trace=False):
    import sys
    if "/opt/trn_rl_repo" not in sys.path:
        sys.path.insert(0, "/opt/trn_rl_repo")
    from concourse.bass_utils import run_bass_kernel_spmd

    if "nc" not in _NC_CACHE:
        _NC_CACHE["nc"] = build_bass()
    nc = _NC_CACHE["nc"]

    in_maps = make_in_maps(x, context, Wq, Wk, Wv, Wo, bo)
    res = run_bass_kernel_spmd(
        nc, in_maps, core_ids=list(range(N_CORES)), trace=_trace)

    out = np.empty((B, NQ, QD), np.float32)
    for c in range(N_CORES):
        b, half = c // 2, c % 2
        out[b, half * NQL:(half + 1) * NQL, :] = \
            np.asarray(res.results[c]["out"], dtype=np.float32)
    if _trace:
        return out, res
    return out
